# revision 17
# baseline (speedup 1.0000x reference)
"""GCN layer kernel for 8 trn2 NeuronCores (SPMD, single launch).

Math:  out = D^-1/2 (A+I) D^-1/2 X W^T + b
     = S A S U + S^2 U + b,   S = diag(s), s = (rowsum(A)+1)^-1/2, U = X W^T

Distribution: row-shard A across 8 cores (strip = 1024 rows = local i's).
Host prep is layout/dtype only (as the baseline already did for X): each
core receives its strip of A^T as a bf16 "SBUF image"
  at_img[p, w*JT*IW + jt*IW + i] = A[c*1024 + w*IW + i, jt*128 + p]
(WV waves over the local i range). No fp32 on-chip transposes; A is
streamed from HBM exactly once as bf16 (16.8MB/core).

Per core:
  stream at_img -> SBUF (NDMA DMAs). While streaming:
    U = X@W^T (64 small matmuls from X^T bf16, drained to SBUF bf16)
    degree: per jt-pair VectorE pair-add halves the data, then a
    ones^T-stationary matmul partition-reduce accumulates deg[1, IW]
    per wave on TensorE.
  per wave w: local s = rsqrt(deg+1); M = ones x s outer (bcast);
    E = UlocT*M^2 + b; AllGather wave-w degrees (the only collectives).
    Wave 0's AllGather overlaps wave 1's streaming; Z matmuls for
    wave-0-unlocked j-tiles overlap wave 1's AllGather.
  Z^T[f,i] = sum_j U[j,f]*s_j*A[i_loc,j]: per (jt, i-region) one matmul
    with 512-wide moving operand, accumulating WV [128,512] PSUM chains.
  outT = Z^T*M + E; output returned transposed; host transposes at gather.
"""

import numpy as np
import ml_dtypes

N = 8192          # nodes
F = 128           # in/out feature dim
NCORES = 8
SR = N // NCORES  # strip rows per core = 1024 (local i's)
P = 128           # partitions / tile edge
JT = N // P       # 64 j tiles (contraction)
WV = 2            # degree/collective waves over the local i range
IW = SR // WV     # i columns per wave = 512
QW = IW // P      # j-tiles unlocked per wave per rank chunk = 4
GR = NCORES * QW  # gathered-degree rows per wave = 32
NDMA = 32         # stream DMAs (512KB each)

_CACHE = {}


def _build_nc():
    import concourse.mybir as mybir
    from concourse import bass
    from concourse.tile import TileContext

    f32 = mybir.dt.float32
    bf16 = mybir.dt.bfloat16
    AF = mybir.ActivationFunctionType

    nc = bass.Bass(num_devices=NCORES)

    At_d = nc.declare_dram_parameter("at_img", [P, JT * SR], bf16, False)
    Xt = nc.declare_dram_parameter("xt_bf", [P, N], bf16, False)      # X^T bf16
    XtL = nc.declare_dram_parameter("xt_loc", [P, SR], bf16, False)   # local cols
    Wt = nc.declare_dram_parameter("wt", [P, F], f32, False)          # W^T
    Bp = nc.declare_dram_parameter("b_pc", [P, 1], f32, False)        # bias col
    Idn = nc.declare_dram_parameter("ident", [P, P], f32, False)
    outT = nc.declare_dram_parameter("outT", [P, SR], f32, True)      # out^T

    sLd = [nc.dram_tensor(f"s_local{w}", [1, IW], f32) for w in range(WV)]
    sAd = [nc.dram_tensor(f"s_all{w}", [GR, P], f32,
                          addr_space="Shared") for w in range(WV)]

    with TileContext(nc) as tc:
        with tc.tile_pool(name="const", bufs=1) as constp, \
             tc.tile_pool(name="big", bufs=1) as bigp, \
             tc.tile_pool(name="small", bufs=1) as smallp, \
             tc.tile_pool(name="vps", bufs=2) as vpsp, \
             tc.tile_pool(name="ups", bufs=4, space="PSUM") as ups, \
             tc.tile_pool(name="degps", bufs=1, space="PSUM") as degps, \
             tc.tile_pool(name="zps", bufs=2, space="PSUM") as zps:

            # ---- constants / small inputs ----
            ident = constp.tile([P, P], f32)
            nc.scalar.dma_start(out=ident[:, :], in_=Idn[:, :])
            wt_sb = constp.tile([P, F], f32)
            nc.scalar.dma_start(out=wt_sb[:, :], in_=Wt[:, :])
            bp_sb = constp.tile([P, 1], f32)
            nc.scalar.dma_start(out=bp_sb[:, :], in_=Bp[:, :])
            wt_bf = constp.tile([P, F], bf16)
            nc.vector.tensor_copy(wt_bf[:, :], wt_sb[:, :])
            ones_c = constp.tile([P, 1], bf16)
            nc.vector.memset(ones_c[:, :], 1.0)
            ones_r = constp.tile([1, P], f32)
            nc.vector.memset(ones_r[:, :], 1.0)


            xt_sb = bigp.tile([P, N], bf16)
            nc.scalar.dma_start(out=xt_sb[:, :], in_=Xt[:, :])
            xtl_sb = constp.tile([P, SR], bf16)
            nc.scalar.dma_start(out=xtl_sb[:, :], in_=XtL[:, :])

            # ---- persistent big buffers ----
            At = bigp.tile([P, JT * SR], bf16)   # A^T strip image
            Up = bigp.tile([P, N], bf16)         # U = X@W^T tiles [j, f]
            UlocT = bigp.tile([P, SR], f32)      # U^T local [f, i] -> E
            M = bigp.tile([P, SR], f32)          # s_i bcast over f
            M2 = bigp.tile([P, SR], f32)
            outT_sb = bigp.tile([P, SR], f32)
            sL = smallp.tile([1, SR], f32)       # local s
            degL_sb = smallp.tile([1, SR], f32)

            # ---- stream A^T image ----
            CDMA = JT * SR // NDMA
            for g in range(NDMA):
                nc.sync.dma_start(
                    out=At[:, g * CDMA:(g + 1) * CDMA],
                    in_=At_d[:, g * CDMA:(g + 1) * CDMA],
                )


            def rsqrt_newton(dst, src, shape):
                # dst = (src+1)^-1/2 with one Newton step to fix sqrt LUT err
                sq = vpsp.tile(shape, f32, tag="nt0", bufs=1)
                nc.scalar.activation(sq, src, AF.Sqrt, bias=1.0)
                r0 = vpsp.tile(shape, f32, tag="nt1", bufs=1)
                nc.vector.reciprocal(r0, sq)
                d1 = vpsp.tile(shape, f32, tag="nt2", bufs=1)
                nc.vector.tensor_scalar_add(d1, src, 1.0)
                t = vpsp.tile(shape, f32, tag="nt3", bufs=1)
                nc.vector.tensor_mul(t, r0, r0)
                nc.vector.tensor_mul(t, t, d1)
                nc.scalar.activation(t, t, AF.Copy, bias=1.5, scale=-0.5)
                nc.vector.tensor_mul(dst, r0, t)

            # ---- per wave: degree reduce -> local rsqrt -> AllGather s.
            # The chain is kept free of other TensorE work so the collective
            # trigger (the critical feeder) fires as early as possible. ----
            for w in range(WV):
                dg = degps.tile([1, IW], f32, tag="dg")
                for k in range(JT // 2):
                    jt0, jt1 = 2 * k, 2 * k + 1
                    vp = vpsp.tile([P, IW], bf16, tag="vp", bufs=8)
                    nc.vector.tensor_add(
                        vp[:, :],
                        At[:, (w * JT + jt0) * IW:(w * JT + jt0 + 1) * IW],
                        At[:, (w * JT + jt1) * IW:(w * JT + jt1 + 1) * IW],
                    )
                    nc.tensor.matmul(
                        dg[:, :], ones_c[:, :], vp[:, :],
                        start=(k == 0), stop=(k == JT // 2 - 1),
                    )
                # high_priority: the scheduler's cost model undervalues the
                # collective latency and would otherwise park this feeder
                # path behind the previous wave's collective round-trip
                with tc.high_priority():
                    nc.scalar.copy(degL_sb[0:1, w * IW:(w + 1) * IW],
                                   dg[:, :])
                    # s = (deg+1)^-1/2 pre-collective; gather s, not deg
                    rsqrt_newton(sL[0:1, w * IW:(w + 1) * IW],
                                 degL_sb[0:1, w * IW:(w + 1) * IW], [1, IW])
                    nc.scalar.dma_start(out=sLd[w][:, :],
                                        in_=sL[0:1, w * IW:(w + 1) * IW])
                    nc.gpsimd.collective_compute(
                        "AllGather", mybir.AluOpType.bypass,
                        replica_groups=[list(range(NCORES))],
                        ins=[sLd[w][:, :]], outs=[sAd[w][:, :]],
                    )

            # ---- U = X@W^T after the degree chains: fills the TensorE
            # through the collective window (doubles as HAM keep-warm) ----
            for ujt in range(JT):
                up_t = ups.tile([P, F], f32, tag="u")
                nc.tensor.matmul(
                    up_t[:, :], xt_sb[:, ujt * P:(ujt + 1) * P],
                    wt_bf[:, :], start=True, stop=True,
                )
                nc.scalar.copy(Up[:, ujt * F:(ujt + 1) * F], up_t[:, :])

            # ---- U^T local [f, i] (wt stationary, xt_loc moving) ----
            for h in range(SR // 512):
                ul_t = ups.tile([P, 512], f32, tag="u")
                nc.tensor.matmul(
                    ul_t[:, :], wt_bf[:, :], xtl_sb[:, h * 512:(h + 1) * 512],
                    start=True, stop=True,
                )
                nc.scalar.copy(UlocT[:, h * 512:(h + 1) * 512], ul_t[:, :])

            # ---- local prep per wave (overlaps the collectives) ----
            for w in range(WV):
                mp = ups.tile([P, IW], f32, tag="u")
                nc.tensor.matmul(
                    mp[:, :], ones_r[:, :], sL[0:1, w * IW:(w + 1) * IW],
                    start=True, stop=True,
                )
                nc.vector.tensor_copy(M[:, w * IW:(w + 1) * IW], mp[:, :])
                nc.vector.tensor_mul(M2[:, w * IW:(w + 1) * IW],
                                     M[:, w * IW:(w + 1) * IW],
                                     M[:, w * IW:(w + 1) * IW])
                # E = UlocT*M2 + b  (in place on UlocT)
                nc.vector.tensor_mul(UlocT[:, w * IW:(w + 1) * IW],
                                     UlocT[:, w * IW:(w + 1) * IW],
                                     M2[:, w * IW:(w + 1) * IW])
                nc.vector.tensor_scalar_add(UlocT[:, w * IW:(w + 1) * IW],
                                            UlocT[:, w * IW:(w + 1) * IW],
                                            bp_sb[:, 0:1])

            # ---- per wave (post-collective): global s, scale Y, Z chains ----
            zp = [zps.tile([P, IW], f32, tag="z", name=f"zp{h}")
                  for h in range(WV)]
            for w in range(WV):
                sG = smallp.tile([GR, P], f32, name=f"sG{w}")
                nc.sync.dma_start(out=sG[:, :], in_=sAd[w][:, :])
                dps = zps.tile([P, GR], f32, tag="tp", bufs=1)
                nc.tensor.transpose(dps[:, :], sG[:, :], ident[0:GR, 0:GR])
                dT = smallp.tile([P, GR], f32, name=f"dinvT{w}")
                nc.vector.tensor_copy(dT[:, :], dps[:, :])

                def scale_up(c, q):
                    jt = c * (JT // NCORES) + w * QW + q
                    k = c * QW + q
                    nc.vector.tensor_scalar_mul(
                        Up[:, jt * F:(jt + 1) * F],
                        Up[:, jt * F:(jt + 1) * F],
                        dT[:, k:k + 1],
                    )
                    return jt

                def zmm(jt, h, start, stop):
                    nc.tensor.matmul(
                        zp[h][:, :],
                        Up[:, jt * F:(jt + 1) * F],
                        At[:, (h * JT + jt) * IW:(h * JT + jt + 1) * IW],
                        start=start, stop=stop,
                    )

                if w < WV - 1:
                    for c in range(NCORES):
                        for q in range(QW):
                            jt = scale_up(c, q)
                            for h in range(WV):
                                zmm(jt, h, start=(c == 0 and q == 0), stop=False)
                else:
                    # final wave: close chain h=0 first so its epilogue and
                    # output DMA overlap chain h=1's tail
                    jts = [scale_up(c, q)
                           for c in range(NCORES) for q in range(QW)]
                    for h in range(WV):
                        for n, jt in enumerate(jts):
                            zmm(jt, h, start=False, stop=(n == len(jts) - 1))
                        nc.vector.tensor_mul(
                            outT_sb[:, h * IW:(h + 1) * IW],
                            zp[h][:, :], M[:, h * IW:(h + 1) * IW])
                        nc.vector.tensor_add(
                            outT_sb[:, h * IW:(h + 1) * IW],
                            outT_sb[:, h * IW:(h + 1) * IW],
                            UlocT[:, h * IW:(h + 1) * IW])
                        nc.scalar.dma_start(
                            out=outT[:, h * IW:(h + 1) * IW],
                            in_=outT_sb[:, h * IW:(h + 1) * IW])

            # ---- epilogue: outT = Z^T*M + E ; DMA out ----
            for h in range(WV):
                nc.vector.tensor_mul(outT_sb[:, h * IW:(h + 1) * IW],
                                     zp[h][:, :], M[:, h * IW:(h + 1) * IW])
                nc.vector.tensor_add(outT_sb[:, h * IW:(h + 1) * IW],
                                     outT_sb[:, h * IW:(h + 1) * IW],
                                     UlocT[:, h * IW:(h + 1) * IW])
                nc.scalar.dma_start(out=outT[:, h * IW:(h + 1) * IW],
                                  in_=outT_sb[:, h * IW:(h + 1) * IW])

    return nc


_NO_SPLIT_TYPES = ("InstEventSemaphore", "InstSemaphore", "InstTrigger")


def _split_drain_waits(nc, max_waits=1):
    """This walrus build only encodes one sem-wait per instruction; hoist
    extras onto preceding same-engine NOPs (monotonic sems => equivalent)."""
    import concourse.mybir as mybir
    for fn in nc.m.functions:
        for blk in fn.blocks:
            newlist = []
            for ins in blk.instructions:
                si = getattr(ins, "sync_info", None)
                tname = type(ins).__name__
                if si is not None and si.on_wait and len(si.on_wait) > max_waits \
                        and not any(tname.startswith(t) for t in _NO_SPLIT_TYPES):
                    waits = list(si.on_wait)
                    for j, w in enumerate(waits[max_waits:]):
                        newlist.append(mybir.InstNoOp(
                            name=f"{ins.name}-w{j}", engine=ins.engine,
                            ins=[], outs=[],
                            sync_info=mybir.SyncInfo(on_wait=[w], on_update=[]),
                        ))
                    si.on_wait = waits[:max_waits]
                newlist.append(ins)
            blk.instructions[:] = newlist


def _get_nc():
    if "nc" not in _CACHE:
        nc = _build_nc()
        _split_drain_waits(nc)
        _CACHE["nc"] = nc
    return _CACHE["nc"]


def _make_in_maps(X, A, W, b):
    bf16 = ml_dtypes.bfloat16
    X = np.ascontiguousarray(np.asarray(X, dtype=np.float32))
    A = np.asarray(A, dtype=np.float32)
    Wm = np.ascontiguousarray(np.asarray(W, dtype=np.float32))
    b = np.ascontiguousarray(np.asarray(b, dtype=np.float32))
    Xt_bf = np.ascontiguousarray(X.T).astype(bf16)
    Wt = np.ascontiguousarray(Wm.T)
    Bp = np.ascontiguousarray(b[:, None])
    Idn = np.eye(P, dtype=np.float32)
    A_bf = A.astype(bf16)
    in_maps = []
    for c in range(NCORES):
        strip = A_bf[c * SR:(c + 1) * SR, :]
        img = np.ascontiguousarray(
            strip.reshape(WV, IW, JT, P).transpose(3, 0, 2, 1)
        ).reshape(P, JT * SR)
        in_maps.append({
            "at_img": img,
            "xt_bf": Xt_bf,
            "xt_loc": np.ascontiguousarray(Xt_bf[:, c * SR:(c + 1) * SR]),
            "wt": Wt,
            "b_pc": Bp,
            "ident": Idn,
        })
    return in_maps


def _install_ntff_hook():
    """This image's antenv lacks axon_hooks; synthesize it so trace=True
    can reach the terminal's NTFF capture via the libaxon ctypes hook."""
    import sys
    import types
    if "antenv.axon_hooks" in sys.modules:
        return
    try:
        from trn_agent_boot.trn_boot import _ntff_profile_via_ctypes
        hook = _ntff_profile_via_ctypes("/opt/axon/libaxon_pjrt.so")
    except Exception:
        hook = None
    mod = types.ModuleType("antenv.axon_hooks")
    mod._hook = hook
    mod.get_axon_ntff_profile_hook = lambda: mod._hook
    def _set(h):
        mod._hook = h
    mod.set_axon_ntff_profile_hook = _set
    sys.modules["antenv.axon_hooks"] = mod
    import antenv
    antenv.axon_hooks = mod
    # the artifact upload needs a bucket this sandbox doesn't have
    import concourse.bass_utils as bu
    bu.upload_artifacts = lambda tmpdir: f"local:{tmpdir}"


def _gather_out(results):
    out = np.empty((N, F), dtype=np.float32)
    for c in range(NCORES):
        out[c * SR:(c + 1) * SR, :] = \
            np.asarray(results[c], dtype=np.float32).T
    return out


def run(X, A, W, b, trace=False, **trace_kwargs):
    """Run on hardware; returns (output, BassKernelResults)."""
    from concourse.bass_utils import run_bass_kernel_spmd
    if trace:
        _install_ntff_hook()
    nc = _get_nc()
    in_maps = _make_in_maps(X, A, W, b)
    res = run_bass_kernel_spmd(nc, in_maps, list(range(NCORES)),
                               trace=trace, **trace_kwargs)
    out = _gather_out([res.results[c]["outT"] for c in range(NCORES)])
    return out, res


def kernel(X, A, W, b):
    out, _ = run(X, A, W, b, trace=False)
    return out


# revision 18
# speedup vs baseline: 1.0351x; 1.0351x over previous
"""GCN layer kernel for 8 trn2 NeuronCores (SPMD, single launch).

Math:  out = D^-1/2 (A+I) D^-1/2 X W^T + b
     = S A S U + S^2 U + b,   S = diag(s), s = (rowsum(A)+1)^-1/2, U = X W^T

Distribution: row-shard A across 8 cores (strip = 1024 rows = local i's).
Host prep is layout/dtype only (as the baseline already did for X): each
core receives its strip of A^T as a bf16 "SBUF image"
  at_img[p, w*JT*IW + jt*IW + i] = A[c*1024 + w*IW + i, jt*128 + p]
(WV waves over the local i range). No fp32 on-chip transposes; A is
streamed from HBM exactly once as bf16 (16.8MB/core).

Per core:
  stream at_img -> SBUF (NDMA DMAs). While streaming:
    U = X@W^T (64 small matmuls from X^T bf16, drained to SBUF bf16)
    degree: per jt-pair VectorE pair-add halves the data, then a
    ones^T-stationary matmul partition-reduce accumulates deg[1, IW]
    per wave on TensorE.
  per wave w: local s = rsqrt(deg+1); M = ones x s outer (bcast);
    E = UlocT*M^2 + b; AllGather wave-w degrees (the only collectives).
    Wave 0's AllGather overlaps wave 1's streaming; Z matmuls for
    wave-0-unlocked j-tiles overlap wave 1's AllGather.
  Z^T[f,i] = sum_j U[j,f]*s_j*A[i_loc,j]: per (jt, i-region) one matmul
    with 512-wide moving operand, accumulating WV [128,512] PSUM chains.
  outT = Z^T*M + E; output returned transposed; host transposes at gather.
"""

import numpy as np
import ml_dtypes

N = 8192          # nodes
F = 128           # in/out feature dim
NCORES = 8
SR = N // NCORES  # strip rows per core = 1024 (local i's)
P = 128           # partitions / tile edge
JT = N // P       # 64 j tiles (contraction)
WV = 2            # degree/collective waves over the local i range
IW = SR // WV     # i columns per wave = 512
QW = IW // P      # j-tiles unlocked per wave per rank chunk = 4
GR = NCORES * QW  # gathered-degree rows per wave = 32
NDMA = 32         # stream DMAs (512KB each)

_CACHE = {}


def _build_nc():
    import concourse.mybir as mybir
    from concourse import bass
    from concourse.tile import TileContext

    f32 = mybir.dt.float32
    bf16 = mybir.dt.bfloat16
    AF = mybir.ActivationFunctionType

    nc = bass.Bass(num_devices=NCORES)

    At_d = nc.declare_dram_parameter("at_img", [P, JT * SR], bf16, False)
    Xt = nc.declare_dram_parameter("xt_bf", [P, N], bf16, False)      # X^T bf16
    XtL = nc.declare_dram_parameter("xt_loc", [P, SR], bf16, False)   # local cols
    Wt = nc.declare_dram_parameter("wt", [P, F], f32, False)          # W^T
    Bp = nc.declare_dram_parameter("b_pc", [P, 1], f32, False)        # bias col
    Idn = nc.declare_dram_parameter("ident", [P, P], f32, False)
    outT = nc.declare_dram_parameter("outT", [P, SR], f32, True)      # out^T

    sLd = [nc.dram_tensor(f"s_local{w}", [1, IW], f32) for w in range(WV)]
    sAd = [nc.dram_tensor(f"s_all{w}", [GR, P], f32,
                          addr_space="Shared") for w in range(WV)]

    with TileContext(nc) as tc:
        with tc.tile_pool(name="const", bufs=1) as constp, \
             tc.tile_pool(name="big", bufs=1) as bigp, \
             tc.tile_pool(name="small", bufs=1) as smallp, \
             tc.tile_pool(name="vps", bufs=2) as vpsp, \
             tc.tile_pool(name="ups", bufs=4, space="PSUM") as ups, \
             tc.tile_pool(name="degps", bufs=1, space="PSUM") as degps, \
             tc.tile_pool(name="zps", bufs=2, space="PSUM") as zps:

            # ---- constants / small inputs ----
            ident = constp.tile([P, P], f32)
            nc.scalar.dma_start(out=ident[:, :], in_=Idn[:, :])
            wt_sb = constp.tile([P, F], f32)
            nc.scalar.dma_start(out=wt_sb[:, :], in_=Wt[:, :])
            bp_sb = constp.tile([P, 1], f32)
            nc.scalar.dma_start(out=bp_sb[:, :], in_=Bp[:, :])
            wt_bf = constp.tile([P, F], bf16)
            nc.vector.tensor_copy(wt_bf[:, :], wt_sb[:, :])
            ones_c = constp.tile([P, 1], bf16)
            nc.vector.memset(ones_c[:, :], 1.0)
            ones_r = constp.tile([1, P], f32)
            nc.vector.memset(ones_r[:, :], 1.0)


            xt_sb = bigp.tile([P, N], bf16)
            nc.scalar.dma_start(out=xt_sb[:, :], in_=Xt[:, :])
            xtl_sb = constp.tile([P, SR], bf16)
            nc.scalar.dma_start(out=xtl_sb[:, :], in_=XtL[:, :])

            # ---- persistent big buffers ----
            At = bigp.tile([P, JT * SR], bf16)   # A^T strip image
            Up = bigp.tile([P, N], bf16)         # U = X@W^T tiles [j, f]
            UlocT = bigp.tile([P, SR], f32)      # U^T local [f, i] -> E
            M = bigp.tile([P, SR], f32)          # s_i bcast over f
            M2 = bigp.tile([P, SR], f32)
            outT_sb = bigp.tile([P, SR], f32)
            sL = smallp.tile([1, SR], f32)       # local s
            degL_sb = smallp.tile([1, SR], f32)

            # ---- stream A^T image ----
            CDMA = JT * SR // NDMA
            for g in range(NDMA):
                nc.sync.dma_start(
                    out=At[:, g * CDMA:(g + 1) * CDMA],
                    in_=At_d[:, g * CDMA:(g + 1) * CDMA],
                )


            def rsqrt_newton(dst, src, shape):
                # dst = (src+1)^-1/2 with one Newton step to fix sqrt LUT err
                sq = vpsp.tile(shape, f32, tag="nt0", bufs=1)
                nc.scalar.activation(sq, src, AF.Sqrt, bias=1.0)
                r0 = vpsp.tile(shape, f32, tag="nt1", bufs=1)
                nc.vector.reciprocal(r0, sq)
                d1 = vpsp.tile(shape, f32, tag="nt2", bufs=1)
                nc.vector.tensor_scalar_add(d1, src, 1.0)
                t = vpsp.tile(shape, f32, tag="nt3", bufs=1)
                nc.vector.tensor_mul(t, r0, r0)
                nc.vector.tensor_mul(t, t, d1)
                nc.scalar.activation(t, t, AF.Copy, bias=1.5, scale=-0.5)
                nc.vector.tensor_mul(dst, r0, t)

            # ---- per wave: degree reduce -> local rsqrt -> AllGather s.
            # The chain is kept free of other TensorE work so the collective
            # trigger (the critical feeder) fires as early as possible. ----
            for w in range(WV):
                dg = degps.tile([1, IW], f32, tag="dg")
                for k in range(JT // 2):
                    jt0, jt1 = 2 * k, 2 * k + 1
                    vp = vpsp.tile([P, IW], bf16, tag="vp", bufs=8)
                    nc.vector.tensor_add(
                        vp[:, :],
                        At[:, (w * JT + jt0) * IW:(w * JT + jt0 + 1) * IW],
                        At[:, (w * JT + jt1) * IW:(w * JT + jt1 + 1) * IW],
                    )
                    nc.tensor.matmul(
                        dg[:, :], ones_c[:, :], vp[:, :],
                        start=(k == 0), stop=(k == JT // 2 - 1),
                    )
                # high_priority: the scheduler's cost model undervalues the
                # collective latency and would otherwise park this feeder
                # path behind the previous wave's collective round-trip
                with tc.high_priority():
                    nc.scalar.copy(degL_sb[0:1, w * IW:(w + 1) * IW],
                                   dg[:, :])
                    # s = (deg+1)^-1/2 pre-collective; gather s, not deg
                    rsqrt_newton(sL[0:1, w * IW:(w + 1) * IW],
                                 degL_sb[0:1, w * IW:(w + 1) * IW], [1, IW])
                    nc.scalar.dma_start(out=sLd[w][:, :],
                                        in_=sL[0:1, w * IW:(w + 1) * IW])
                    nc.gpsimd.collective_compute(
                        "AllGather", mybir.AluOpType.bypass,
                        replica_groups=[list(range(NCORES))],
                        ins=[sLd[w][:, :]], outs=[sAd[w][:, :]],
                    )

            # ---- U = X@W^T after the degree chains: fills the TensorE
            # through the collective window (doubles as HAM keep-warm) ----
            for ujt in range(JT):
                up_t = ups.tile([P, F], f32, tag="u")
                nc.tensor.matmul(
                    up_t[:, :], xt_sb[:, ujt * P:(ujt + 1) * P],
                    wt_bf[:, :], start=True, stop=True,
                )
                nc.scalar.copy(Up[:, ujt * F:(ujt + 1) * F], up_t[:, :])

            # ---- U^T local [f, i] (wt stationary, xt_loc moving) ----
            for h in range(SR // 512):
                ul_t = ups.tile([P, 512], f32, tag="u")
                nc.tensor.matmul(
                    ul_t[:, :], wt_bf[:, :], xtl_sb[:, h * 512:(h + 1) * 512],
                    start=True, stop=True,
                )
                nc.scalar.copy(UlocT[:, h * 512:(h + 1) * 512], ul_t[:, :])

            # ---- local prep per wave (overlaps the collectives) ----
            for w in range(WV):
                mp = ups.tile([P, IW], f32, tag="u")
                nc.tensor.matmul(
                    mp[:, :], ones_r[:, :], sL[0:1, w * IW:(w + 1) * IW],
                    start=True, stop=True,
                )
                nc.vector.tensor_copy(M[:, w * IW:(w + 1) * IW], mp[:, :])
                nc.vector.tensor_mul(M2[:, w * IW:(w + 1) * IW],
                                     M[:, w * IW:(w + 1) * IW],
                                     M[:, w * IW:(w + 1) * IW])
                # E = UlocT*M2 + b  (in place on UlocT)
                nc.vector.tensor_mul(UlocT[:, w * IW:(w + 1) * IW],
                                     UlocT[:, w * IW:(w + 1) * IW],
                                     M2[:, w * IW:(w + 1) * IW])
                nc.vector.tensor_scalar_add(UlocT[:, w * IW:(w + 1) * IW],
                                            UlocT[:, w * IW:(w + 1) * IW],
                                            bp_sb[:, 0:1])

            # ---- per wave (post-collective): global s, scale Y, Z chains ----
            zp = [zps.tile([P, IW], f32, tag="z", name=f"zp{h}")
                  for h in range(WV)]
            for w in range(WV):
                # tile_wait_until: logical-time marker so the scheduler
                # orders all pre-collective work (esp. the wave-1 feeder)
                # ahead of this block on every engine queue -- its cost
                # model assumes collectives are fast, which they are not
                ctx_w = tc.tile_wait_until(1.0 + w)
                ctx_w.__enter__()
                sG = smallp.tile([GR, P], f32, name=f"sG{w}")
                nc.sync.dma_start(out=sG[:, :], in_=sAd[w][:, :])
                dps = zps.tile([P, GR], f32, tag="tp", bufs=1)
                nc.tensor.transpose(dps[:, :], sG[:, :], ident[0:GR, 0:GR])
                dT = smallp.tile([P, GR], f32, name=f"dinvT{w}")
                nc.vector.tensor_copy(dT[:, :], dps[:, :])

                def scale_up(c, q):
                    jt = c * (JT // NCORES) + w * QW + q
                    k = c * QW + q
                    nc.vector.tensor_scalar_mul(
                        Up[:, jt * F:(jt + 1) * F],
                        Up[:, jt * F:(jt + 1) * F],
                        dT[:, k:k + 1],
                    )
                    return jt

                def zmm(jt, h, start, stop):
                    nc.tensor.matmul(
                        zp[h][:, :],
                        Up[:, jt * F:(jt + 1) * F],
                        At[:, (h * JT + jt) * IW:(h * JT + jt + 1) * IW],
                        start=start, stop=stop,
                    )

                if w < WV - 1:
                    for c in range(NCORES):
                        for q in range(QW):
                            jt = scale_up(c, q)
                            for h in range(WV):
                                zmm(jt, h, start=(c == 0 and q == 0), stop=False)
                    ctx_w.__exit__(None, None, None)
                else:
                    # final wave: close chain h=0 first so its epilogue and
                    # output DMA overlap chain h=1's tail
                    jts = [scale_up(c, q)
                           for c in range(NCORES) for q in range(QW)]
                    for h in range(WV):
                        for n, jt in enumerate(jts):
                            zmm(jt, h, start=False, stop=(n == len(jts) - 1))
                        nc.vector.tensor_mul(
                            outT_sb[:, h * IW:(h + 1) * IW],
                            zp[h][:, :], M[:, h * IW:(h + 1) * IW])
                        nc.vector.tensor_add(
                            outT_sb[:, h * IW:(h + 1) * IW],
                            outT_sb[:, h * IW:(h + 1) * IW],
                            UlocT[:, h * IW:(h + 1) * IW])
                        nc.scalar.dma_start(
                            out=outT[:, h * IW:(h + 1) * IW],
                            in_=outT_sb[:, h * IW:(h + 1) * IW])
                ctx_w.__exit__(None, None, None)

            # ---- epilogue: outT = Z^T*M + E ; DMA out ----
            for h in range(WV):
                nc.vector.tensor_mul(outT_sb[:, h * IW:(h + 1) * IW],
                                     zp[h][:, :], M[:, h * IW:(h + 1) * IW])
                nc.vector.tensor_add(outT_sb[:, h * IW:(h + 1) * IW],
                                     outT_sb[:, h * IW:(h + 1) * IW],
                                     UlocT[:, h * IW:(h + 1) * IW])
                nc.scalar.dma_start(out=outT[:, h * IW:(h + 1) * IW],
                                  in_=outT_sb[:, h * IW:(h + 1) * IW])

    return nc


_NO_SPLIT_TYPES = ("InstEventSemaphore", "InstSemaphore", "InstTrigger")


def _split_drain_waits(nc, max_waits=1):
    """This walrus build only encodes one sem-wait per instruction; hoist
    extras onto preceding same-engine NOPs (monotonic sems => equivalent)."""
    import concourse.mybir as mybir
    for fn in nc.m.functions:
        for blk in fn.blocks:
            newlist = []
            for ins in blk.instructions:
                si = getattr(ins, "sync_info", None)
                tname = type(ins).__name__
                if si is not None and si.on_wait and len(si.on_wait) > max_waits \
                        and not any(tname.startswith(t) for t in _NO_SPLIT_TYPES):
                    waits = list(si.on_wait)
                    for j, w in enumerate(waits[max_waits:]):
                        newlist.append(mybir.InstNoOp(
                            name=f"{ins.name}-w{j}", engine=ins.engine,
                            ins=[], outs=[],
                            sync_info=mybir.SyncInfo(on_wait=[w], on_update=[]),
                        ))
                    si.on_wait = waits[:max_waits]
                newlist.append(ins)
            blk.instructions[:] = newlist


def _get_nc():
    if "nc" not in _CACHE:
        nc = _build_nc()
        _split_drain_waits(nc)
        _CACHE["nc"] = nc
    return _CACHE["nc"]


def _make_in_maps(X, A, W, b):
    bf16 = ml_dtypes.bfloat16
    X = np.ascontiguousarray(np.asarray(X, dtype=np.float32))
    A = np.asarray(A, dtype=np.float32)
    Wm = np.ascontiguousarray(np.asarray(W, dtype=np.float32))
    b = np.ascontiguousarray(np.asarray(b, dtype=np.float32))
    Xt_bf = np.ascontiguousarray(X.T).astype(bf16)
    Wt = np.ascontiguousarray(Wm.T)
    Bp = np.ascontiguousarray(b[:, None])
    Idn = np.eye(P, dtype=np.float32)
    A_bf = A.astype(bf16)
    in_maps = []
    for c in range(NCORES):
        strip = A_bf[c * SR:(c + 1) * SR, :]
        img = np.ascontiguousarray(
            strip.reshape(WV, IW, JT, P).transpose(3, 0, 2, 1)
        ).reshape(P, JT * SR)
        in_maps.append({
            "at_img": img,
            "xt_bf": Xt_bf,
            "xt_loc": np.ascontiguousarray(Xt_bf[:, c * SR:(c + 1) * SR]),
            "wt": Wt,
            "b_pc": Bp,
            "ident": Idn,
        })
    return in_maps


def _install_ntff_hook():
    """This image's antenv lacks axon_hooks; synthesize it so trace=True
    can reach the terminal's NTFF capture via the libaxon ctypes hook."""
    import sys
    import types
    if "antenv.axon_hooks" in sys.modules:
        return
    try:
        from trn_agent_boot.trn_boot import _ntff_profile_via_ctypes
        hook = _ntff_profile_via_ctypes("/opt/axon/libaxon_pjrt.so")
    except Exception:
        hook = None
    mod = types.ModuleType("antenv.axon_hooks")
    mod._hook = hook
    mod.get_axon_ntff_profile_hook = lambda: mod._hook
    def _set(h):
        mod._hook = h
    mod.set_axon_ntff_profile_hook = _set
    sys.modules["antenv.axon_hooks"] = mod
    import antenv
    antenv.axon_hooks = mod
    # the artifact upload needs a bucket this sandbox doesn't have
    import concourse.bass_utils as bu
    bu.upload_artifacts = lambda tmpdir: f"local:{tmpdir}"


def _gather_out(results):
    out = np.empty((N, F), dtype=np.float32)
    for c in range(NCORES):
        out[c * SR:(c + 1) * SR, :] = \
            np.asarray(results[c], dtype=np.float32).T
    return out


def run(X, A, W, b, trace=False, **trace_kwargs):
    """Run on hardware; returns (output, BassKernelResults)."""
    from concourse.bass_utils import run_bass_kernel_spmd
    if trace:
        _install_ntff_hook()
    nc = _get_nc()
    in_maps = _make_in_maps(X, A, W, b)
    res = run_bass_kernel_spmd(nc, in_maps, list(range(NCORES)),
                               trace=trace, **trace_kwargs)
    out = _gather_out([res.results[c]["outT"] for c in range(NCORES)])
    return out, res


def kernel(X, A, W, b):
    out, _ = run(X, A, W, b, trace=False)
    return out
